# revision 5
# baseline (speedup 1.0000x reference)
"""BatchConv2D (per-sample-weight conv) Trainium2 Bass kernel.

Problem: x [16,4,64,64,64], weight [16,128,64,3,3], bias [16,128] (all f32)
out[bi,bj] = conv2d(x[bi,bj], weight[bi], pad=1) + bias[bi]  -> [16,4,128,64,64]

Sharding: b_i axis split across 8 cores (2 per core); no communication.

Per-core kernel strategy ("bf16 image-pair" conv-as-matmul):
  - Two images (same b_i, two b_j) share SBUF partitions: channels of image
    A in partitions 0-63, image B in 64-127, stored zero-padded [128,66,66]
    bf16 so each pair loads as one contiguous 1.1MB DMA.
  - Each 8-row output block of one image is a single 9-tap K=64 PSUM
    accumulation chain: out[8,64] += w[tap].T @ x[rows+ky, kx:kx+64].
  - Matmuls are emitted in same-weight ADJACENT-ROW pairs (block b0 then b1
    of the same image/tap), alternating image halves pair-by-pair
    (A,A,B,B). On TRN2 each such bf16 pair streams as one fused 128x1024
    moving operand at 2 bf16 cols/cycle - full 16384 MACs/cycle from K=64
    matmuls. (Measured: breaking dtype to f32r, adjacency, or the per-MM
    LDWEIGHTS pairing each costs 1.6-1.9x.)
  - PSUM: one 2-bank tile per image-half per 2-block set, double buffered
    (8 banks total). DVE drains a whole tile (+bias, f32->bf16) in one op.
  - bf16 I/O halves HBM traffic (13.5MB/core, ~38us) so the kernel stays
    compute-bound; host rounds inputs to bf16 and upconverts the bf16
    output (adds ~3.5e-3 max rel err vs the 2e-2 gate).
  - First image's load is split in two so the PE starts ~2us earlier.
"""

import numpy as np

B_I, B_J, C, H, W = 16, 4, 64, 64, 64
OC, KH, KW = 128, 3, 3
N_CORES = 8
BPC = B_I // N_CORES          # b_i per core (2)
NIMG = BPC * B_J              # images per core (8)
NPAIR = NIMG // 2             # image pairs per core (4)
HP, WPP = H + 2, W + 2        # padded 66x66
BLK = 8                       # output rows per PSUM bank
BPS = 2                       # blocks per set (fused matmul pair)
NSET = H // BLK // BPS        # 4 sets per image pair

_CACHE = {}


def _build_nc(repeat=1, timing=False):
    import concourse.mybir as mybir
    from concourse import bacc, tile

    F32 = mybir.dt.float32
    BF16 = mybir.dt.bfloat16

    io_kind = "Internal" if timing else None
    nc = bacc.Bacc("TRN2", target_bir_lowering=False, debug=False)
    x_d = nc.dram_tensor("x", [NPAIR, 2 * C, HP, WPP], BF16,
                         kind=io_kind or "ExternalInput")
    wt_d = nc.dram_tensor("wt", [BPC, 2 * C, KH * KW, OC], BF16,
                          kind="ExternalInput")
    b_d = nc.dram_tensor("bias", [OC, BPC], F32, kind="ExternalInput")
    o_d = nc.dram_tensor("out", [BPC, B_J, OC, H, W], BF16,
                         kind=io_kind or "ExternalOutput")
    tok_d = (
        nc.dram_tensor("tok", [1, 1], F32, kind="ExternalOutput") if timing
        else None
    )

    with tile.TileContext(nc) as tc:
        with (
            tc.tile_pool(name="const", bufs=1) as cpool,
            tc.tile_pool(name="img", bufs=1) as ipool,
            tc.tile_pool(name="osb", bufs=1) as opool,
            tc.tile_pool(name="ps", bufs=1, space="PSUM") as pspool,
        ):
            wt_t = [
                cpool.tile([2 * C, KH * KW, OC], BF16, name=f"wt{bi}",
                           tag=f"wt{bi}")
                for bi in range(BPC)
            ]
            bias_t = cpool.tile([OC, BPC], F32, name="bias_t", tag="bias")

            def load_img(p, img, split):
                if split:
                    m = HP // 2
                    nc.sync.dma_start(img[:, :m, :], x_d[p, :, :m, :])
                    nc.sync.dma_start(img[:, m:, :], x_d[p, :, m:, :])
                else:
                    nc.sync.dma_start(img[:, :, :], x_d[p])

            def warmup():
                # Dummy matmuls on scratch data during the initial DMA wait:
                # keeps TensorE busy from t=0 so the HAM clock-gate reaches
                # 8/8 (2.4GHz) before the first real matmul (~3.4us window).
                # Results land in a ps bank that the first chain overwrites
                # with start=True; never read.
                warm = cpool.tile([C, 640], BF16, name="warm", tag="warm")
                nc.vector.memset(warm[:, :], 0.0)
                wps = pspool.tile([OC, BLK, W], F32, name="ps00", tag="ps00",
                                  bufs=2)
                for _ in range(8):
                    nc.tensor.matmul(
                        wps[:, :, :], warm[:, 0:OC], warm[:, OC : OC + 512],
                        start=True, stop=True,
                    )

            def body(first):
                for p in range(NPAIR):
                    bi = p // 2
                    bjb = 2 * (p % 2)
                    img = ipool.tile([2 * C, HP, WPP], BF16, name="img",
                                     tag="img", bufs=3)
                    if first and p == 0:
                        # pipelined preamble: wt0 on the sync ring and the
                        # first image rows on the (otherwise idle) scalar
                        # ring in parallel, so their fixed DMA costs
                        # overlap and the PE starts as early as possible
                        m = HP // 2
                        nc.scalar.dma_start(img[:, :m, :], x_d[0, :, :m, :])
                        nc.sync.dma_start(wt_t[0][:, :, :], wt_d[0])
                        nc.sync.dma_start(img[:, m:, :], x_d[0, :, m:, :])
                        nc.sync.dma_start(wt_t[1][:, :, :], wt_d[1])
                        nc.sync.dma_start(bias_t[:, :], b_d[:, :])
                        warmup()
                    else:
                        load_img(p, img, split=False)

                    for s in range(NSET):
                        y0 = s * BLK * BPS
                        pst = {
                            (h, b): pspool.tile([OC, BLK, W], F32,
                                                name=f"ps{h}{b}",
                                                tag=f"ps{h}{b}", bufs=2)
                            for h in range(2)
                            for b in range(BPS)
                        }
                        osb = opool.tile([OC, 2, BLK * BPS, W], BF16,
                                         name="osb", tag="osb", bufs=3)
                        for t in range(KH * KW):
                            ky, kx = t // KW, t % KW
                            for h in range(2):
                                for b in range(BPS):
                                    yb = y0 + BLK * b
                                    nc.tensor.matmul(
                                        pst[(h, b)][:, :, :],
                                        wt_t[bi][64 * h : 64 * h + 64, t, :],
                                        img[
                                            64 * h : 64 * h + 64,
                                            yb + ky : yb + ky + BLK,
                                            kx : kx + W,
                                        ],
                                        start=(t == 0),
                                        stop=(t == KH * KW - 1),
                                    )
                        for h in range(2):
                            for b in range(BPS):
                                nc.vector.tensor_scalar_add(
                                    osb[:, h, BLK * b : BLK * (b + 1), :],
                                    pst[(h, b)][:, :, :],
                                    bias_t[:, bi : bi + 1],
                                )
                        for h in range(2):
                            nc.scalar.dma_start(
                                o_d[bi, bjb + h, :, y0 : y0 + BLK * BPS, :],
                                osb[:, h, :, :],
                            )

            if repeat == 1:
                body(first=True)
            else:
                nc.sync.dma_start(wt_t[0][:, :, :], wt_d[0])
                nc.sync.dma_start(wt_t[1][:, :, :], wt_d[1])
                nc.sync.dma_start(bias_t[:, :], b_d[:, :])
                with tc.For_i(0, repeat):
                    body(first=False)
            if timing:
                tok = cpool.tile([1, 1], F32, name="tok", tag="tok")
                nc.vector.memset(tok[:, :], 1.0)
                nc.scalar.dma_start(tok_d[:, :], tok[:, :])
    nc.compile()
    return nc


def _pack(x, weight, bias):
    """Host-side repack into the kernel's DMA-friendly bf16 layouts."""
    import ml_dtypes

    bf16 = ml_dtypes.bfloat16
    x = np.ascontiguousarray(x, dtype=np.float32)
    weight = np.ascontiguousarray(weight, dtype=np.float32)
    bias = np.ascontiguousarray(bias, dtype=np.float32)

    xp = np.zeros((B_I, B_J, C, HP, WPP), bf16)
    xp[:, :, :, 1 : H + 1, 1 : W + 1] = x.astype(bf16)
    xq = xp.reshape(B_I, 2, 2 * C, HP, WPP)  # [bi, pair, 2C, 66, 66]

    wt0 = np.ascontiguousarray(np.transpose(weight, (0, 2, 3, 4, 1))).reshape(
        B_I, C, KH * KW, OC
    )
    wt = np.concatenate([wt0, wt0], axis=1).astype(bf16)
    bp = np.ascontiguousarray(np.transpose(bias, (1, 0)))  # [OC, B_I]
    return xq, wt, bp


def make_in_maps(xq, wt, bp):
    in_maps = []
    for i in range(N_CORES):
        sl = slice(i * BPC, (i + 1) * BPC)
        in_maps.append(
            {
                "x": np.ascontiguousarray(xq[sl].reshape(NPAIR, 2 * C, HP, WPP)),
                "wt": np.ascontiguousarray(wt[sl]),
                "bias": np.ascontiguousarray(bp[:, sl]),
            }
        )
    return in_maps


def kernel(x, weight, bias):
    from concourse.bass_utils import run_bass_kernel_spmd

    xq, wt, bp = _pack(x, weight, bias)

    if "nc" not in _CACHE:
        _CACHE["nc"] = _build_nc()
    nc = _CACHE["nc"]

    in_maps = make_in_maps(xq, wt, bp)

    res = run_bass_kernel_spmd(nc, in_maps, list(range(N_CORES)))
    out = np.concatenate(
        [np.asarray(res.results[i]["out"]) for i in range(N_CORES)], axis=0
    )
    return out.astype(np.float32)
